# revision 46
# baseline (speedup 1.0000x reference)
"""Trainium2 Bass kernel for the GBM sampling-loss problem.

Contract: kernel(**inputs) takes the FULL unsharded inputs
  x[2,500,3,128,128] z[2,3,128,128] Wm[6,3,3,3] bm[6] temb_w[6] t[2]
and returns the scalar loss (np.float32, shape ()).

Strategy (data parallel over batch x h-quarters = 8 shards):
  - Host slices the 7-step window at t (with dynamic_slice clamping),
    transposes shards to a pixels-on-partitions layout [128w, 32h, 3c, ...],
    builds im2col patches (27 taps + a ones row that folds conv bias +
    time embedding) for the 3x3 SAME conv, and pre-scales z by 2*sqrt(t).
  - Window/patches/z ship in fp16 (inputs only; all arithmetic fp32
    internally) to halve the serial DMA transfer time; the resulting
    quantization noise is ~1e-3 relative, well inside tolerance.
  - Each core: window mean/var stats, conv via 32 tiny matmuls
    (lhsT = patch slice [28,128], rhs = weights [28,6]) landing directly
    in [128w, 32h, 6o] layout, pointwise KL + sampling math, free-dim
    reductions to [128, 4] partial sums.
  - Host combines 8x128 partial sums into the final scalar.

Engine layout:
  SP ring    : win(+wz) DMA in, out DMA
  Pool SWDGE : pat DMA in; Pool engine: TT-only elementwise chains
  ACT        : single table load (natural_log_exp_and_others), Square,
               psum copy, 2x Ln (with accum), Exp(scale=0.5)
  PE         : 32 matmuls
  DVE        : reduces, stats chain, reciprocal, fused r-sum

All t-dependence flows through input *data* so a single SPMD program
serves all 8 cores.
"""

import os
import sys

sys.path.insert(0, "/opt/trn_rl_repo")

import numpy as np

K = 3
T = 500
C = 3
B = 2
H = 128
W = 128
EPS = 1e-7
N_CORES = 8
HS = H // 4  # 32 rows per core
# patch partition-strips: fp16 stationary loads crash when the base
# partition changes between matmuls, so all matmuls read partitions 0:112
# (4 strips of 28 tap-rows stacked at 0/28/56/84) and strip selection
# happens through 4 zero-masked weight-column variants.
NSTRIP = 4
HSTRIP = 8
N_TOT = B * C * H * W  # 98304 elements in the loss means
PATW = HSTRIP * 130  # 1040 patch columns per strip
# 3 rhs weight variants of 9 output channels (6 conv outs + 3 "2*p_mu"
# channels), each starting 4-element aligned for the fp16 matmul
RHSW = 12
LN6 = float(np.log(6.0))
WIN_COLS = HS * C * 7  # 672
ALL_COLS = WIN_COLS + HS * C  # +96 cols of 2*sqrt(t)*z

_built = None  # cached compiled program
LAST_RESULTS = None  # BassKernelResults of the most recent run


def _build_nc():
    import concourse.bacc as bacc
    import concourse.mybir as mybir
    from concourse import tile as tile_mod
    from concourse.tile import add_dep_helper

    f32 = mybir.dt.float32
    f16 = mybir.dt.float16
    bf16 = mybir.dt.bfloat16
    AF = mybir.ActivationFunctionType
    ALU = mybir.AluOpType
    AX = mybir.AxisListType

    nc = bacc.Bacc()

    # window (h,c,s layout) and 2*sqrt(t)*z (h,c) share one fp16 tensor/DMA
    win_d = nc.dram_tensor("win", [128, ALL_COLS], bf16, kind="ExternalInput")
    pat_d = nc.dram_tensor("pat", [112, PATW + NSTRIP * RHSW], f16, kind="ExternalInput")
    out_d = nc.dram_tensor("out", [128, 4], f32, kind="ExternalOutput")

    with tile_mod.TileContext(nc) as tc:
        with (
            tc.tile_pool(name="sb", bufs=1) as sb,
            tc.tile_pool(name="ps", bufs=1, space="PSUM") as ps,
        ):
            wina = sb.tile([128, ALL_COLS], bf16)
            pat = sb.tile([112, PATW + NSTRIP * RHSW], f16)
            out_sb = sb.tile([128, 4], f32)

            nc.sync.dma_start(out=wina[:], in_=win_d[:])    # SP HWDGE
            nc.sync.dma_start(out=pat[:], in_=pat_d[:])     # SP HWDGE
            win = wina[:, 0:WIN_COLS].rearrange(
                "p (h c s) -> p h c s", h=HS, c=C, s=7
            )
            wz = wina[:, WIN_COLS:ALL_COLS].rearrange(
                "p (h c) -> p h c", h=HS, c=C
            )
            HH = HS // 2

            # ---- conv: err[w, h, o], bias+temb folded into ones-row ----
            # K=84 spans all 3 strips; rhs variant s has the weights only
            # in partitions 28s:28s+28 (zeros elsewhere) so each matmul
            # picks its strip without changing the stationary base
            # partition (fp16 loads crash on base-partition switches).
            err_ps = ps.tile([128, HS, 9], f32)
            for h in range(HS):
                s, hh = divmod(h, HSTRIP)
                nc.tensor.matmul(
                    err_ps[:, h, :],
                    pat[0:112, hh * 130 : hh * 130 + 128],
                    pat[0:112, PATW + RHSW * s : PATW + RHSW * s + 9],
                )
            err = sb.tile([128, HS, 9], f32)
            # psg (cols 3:6) feeds the critical sg->sg2->lnS chain; copy it
            # first so that chain starts before the rest of the copy ends
            nc.scalar.copy(err[:, :, 3:6], err_ps[:, :, 3:6])
            errv = err.rearrange("p h (g o) -> p h g o", g=3)
            errpv = err_ps.rearrange("p h (g o) -> p h g o", g=3)
            nc.scalar.copy(errv[:, :, 0:3:2, :], errpv[:, :, 0:3:2, :])
            pm = err[:, :, 0:3]
            psg = err[:, :, 3:6]
            pm2 = err[:, :, 6:9]  # 2*p_mu, free from the PE

            # ---- window stats (intermediates all f32) ----
            musum = sb.tile([128, HS, C], f32)
            nc.vector.tensor_reduce(musum[:], win, axis=AX.X, op=ALU.add)
            sq = sb.tile([128, HS, C, 7], f32)
            nc.scalar.activation(sq[:], win, AF.Square)
            # sum over the 7 slabs on the (otherwise idle) Pool engine so
            # the DVE only runs one of the two big reductions; high
            # priority so the scheduler doesn't queue errS-dependent Pool
            # ops ahead of this tree (Pool executes in-order)
            with tc.high_priority():
                u3 = sb.tile([128, HS, C, 3], f32)
                nc.gpsimd.tensor_tensor(
                    u3[:], sq[:, :, :, 0:6:2], sq[:, :, :, 1:7:2], op=ALU.add
                )
                w1 = sb.tile([128, HS, C], f32)
                nc.gpsimd.tensor_tensor(
                    w1[:], u3[:, :, :, 0], u3[:, :, :, 1], op=ALU.add
                )
                w2 = sb.tile([128, HS, C], f32)
                nc.gpsimd.tensor_tensor(w2[:], w1[:], u3[:, :, :, 2], op=ALU.add)
                ssq = sb.tile([128, HS, C], f32)
                i_ssq = nc.gpsimd.tensor_tensor(ssq[:], w2[:], sq[:, :, :, 6], op=ALU.add)
            mu = sb.tile([128, HS, C], f32)
            nc.vector.tensor_scalar_mul(mu[:], musum[:], 1.0 / 7.0)
            bt = sb.tile([128, HS, C], f32)  # musum^2/7
            nc.vector.scalar_tensor_tensor(
                bt[:], musum[:], 1.0 / 7.0, musum[:], op0=ALU.mult, op1=ALU.mult
            )
            v6 = sb.tile([128, HS, C], f32)  # 6 * unbiased variance
            nc.vector.tensor_tensor(v6[:], ssq[:], bt[:], op=ALU.subtract)
            g6 = sb.tile([128, HS, C], f32)  # max(6*var, 6*EPS^2)
            nc.vector.tensor_scalar_max(g6[:], v6[:], 6.0 * EPS * EPS)
            inv = sb.tile([128, HS, C], f32)  # 1 / (6*gvar)
            i_recip = nc.vector.reciprocal(inv[:], g6[:])

            # ---- KL pieces ----
            sg = sb.tile([128, HS, C], f32)
            i_sg = nc.vector.tensor_scalar_max(sg[:], psg, EPS)
            sg2 = sb.tile([128, HS, C], f32)
            nc.gpsimd.tensor_tensor(sg2[:], sg[:], sg[:], op=ALU.mult)
            dmu = sb.tile([128, HS, C], f32)
            i_dmu = nc.gpsimd.tensor_tensor(dmu[:], pm, mu[:], op=ALU.subtract)
            dmu2 = sb.tile([128, HS, C], f32)
            nc.gpsimd.tensor_tensor(dmu2[:], dmu[:], dmu[:], op=ALU.mult)
            num = sb.tile([128, HS, C], f32)
            nc.gpsimd.tensor_tensor(num[:], sg2[:], dmu2[:], op=ALU.add)
            # ---- sampling branch (Pool engine; TT-only ops there) ----
            # wz holds 2*sqrt(t)*z, the *0.5 folds into Exp's scale:
            # exp arg = 0.5*(2*sg*W + 2*pm + sg^2)
            t1 = sb.tile([128, HS, C], f32)
            nc.gpsimd.tensor_tensor(t1[:], sg[:], wz, op=ALU.mult)
            q1 = sb.tile([128, HS, C], f32)
            nc.gpsimd.tensor_tensor(q1[:], t1[:], sg2[:], op=ALU.add)
            ein = sb.tile([128, HS, C], f32)
            nc.gpsimd.tensor_tensor(ein[:], q1[:], pm2, op=ALU.add)
            e = sb.tile([128, HS, C], f32)
            nc.scalar.activation(e[:], ein[:], AF.Exp, scale=0.5)
            xt = sb.tile([128, HS, C], f32)
            nc.gpsimd.tensor_tensor(xt[:], e[:], win[:, :, :, 2], op=ALU.mult)
            d = sb.tile([128, HS, C], f32)
            nc.gpsimd.tensor_tensor(d[:], xt[:], win[:, :, :, 3], op=ALU.subtract)

            # sum r = sum (sg2 + dmu2) * 6 / g6 ; fused mul+mul+row-sum
            r = sb.tile([128, HS, C], f32)
            nc.vector.scalar_tensor_tensor(
                r[:], num[:], 6.0, inv[:], op0=ALU.mult, op1=ALU.mult,
                accum_out=out_sb[:, 1:2],
            )
            # sum ln(var_ratio) = sum ln(sg2) - sum ln(g6) + N*ln6 (host)
            lnG = sb.tile([128, HS, C], f32)
            nc.scalar.activation(lnG[:], g6[:], AF.Ln, accum_out=out_sb[:, 3:4])
            nc.vector.tensor_reduce(
                out_sb[:, 0:1], d[:], axis=AX.XY, op=ALU.add,
                apply_absolute_value=True,
            )
            lnS = sb.tile([128, HS, C], f32)
            nc.scalar.activation(lnS[:], sg2[:], AF.Ln, accum_out=out_sb[:, 2:3])

            nc.sync.dma_start(out=out_d[:], in_=out_sb[:])

    # Steer insert_act_table_loads to the one set that covers
    # {ln, exp, square, copy}: natural_log_exp_and_others. The dict must
    # keep the original entry order (index == act_func_set_id), so blank
    # the other sets rather than dropping them.
    orig_tables = bacc.get_activation_tables

    def _patched(arch):
        tabs = orig_tables(arch)
        keep = "natural_log_exp_and_others"
        assert keep in tabs
        return {k: (v if k == keep else set()) for k, v in tabs.items()}

    bacc.get_activation_tables = _patched
    try:
        nc.compile()
    finally:
        bacc.get_activation_tables = orig_tables
    return nc


def _prep_inputs(x, z, Wm, bm, temb_w, t):
    """Build the 8 per-core input dicts (pure numpy, host side)."""
    x = np.ascontiguousarray(np.asarray(x, dtype=np.float32))
    z = np.asarray(z, dtype=np.float32)
    Wm = np.asarray(Wm, dtype=np.float32)
    bm = np.asarray(bm, dtype=np.float32)
    temb_w = np.asarray(temb_w, dtype=np.float32)
    t = np.asarray(t)
    try:
        import ml_dtypes
        npbf16 = np.dtype(ml_dtypes.bfloat16)
    except ImportError:  # fall back to jax's dtype
        import jax.numpy as jnp
        npbf16 = np.dtype(jnp.bfloat16)

    wk27 = Wm.transpose(2, 3, 1, 0).reshape(27, 6)  # [(dy,dx,c), o]

    in_maps = []
    for i in range(B):
        ti = int(t[i])
        st = min(max(ti - K, 0), T - (2 * K + 1))  # lax.dynamic_slice clamping
        win = x[i, st : st + 2 * K + 1]  # [7,3,128,128]
        xin = win[K - 1]  # [3,128,128]
        xp = np.zeros((C, H + 2, W + 4), np.float32)
        xp[:, 1 : H + 1, 1 : W + 1] = xin

        bias = bm + temb_w * (np.float32(ti) / np.float32(T))
        wk = np.empty((28, 9), np.float32)
        wk[:27, 0:6] = wk27
        wk[27, 0:6] = bias
        wk[:, 6:9] = 2.0 * wk[:, 0:3]  # "2*p_mu" channels
        sqt2 = np.float32(2.0 * np.sqrt(np.float64(ti)))

        for q in range(4):
            r0 = q * HS
            winT = win[:, :, r0 : r0 + HS, :].transpose(3, 2, 1, 0)  # [w,h,c,s]
            wz = (sqt2 * z[i, :, r0 : r0 + HS, :]).transpose(2, 1, 0)  # [w,h,c]
            wina = np.empty((128, ALL_COLS), dtype=npbf16)
            wina[:, 0:WIN_COLS] = winT.reshape(128, WIN_COLS).astype(npbf16)
            wina[:, WIN_COLS:] = wz.reshape(128, HS * C).astype(npbf16)

            pat = np.zeros((112, PATW + NSTRIP * RHSW), np.float32)
            for s in range(NSTRIP):
                rs = r0 + s * HSTRIP
                for dy in range(3):
                    for dx in range(3):
                        for c in range(C):
                            p = (dy * 3 + dx) * 3 + c
                            pat[28 * s + p, :PATW] = xp[
                                c, rs + dy : rs + dy + HSTRIP, dx : dx + 130
                            ].reshape(-1)
                pat[28 * s + 27, :PATW] = 1.0
                pat[28 * s : 28 * s + 28, PATW + RHSW * s : PATW + RHSW * s + 9] = wk
            in_maps.append({"win": wina, "pat": pat.astype(np.float16)})
    return in_maps


def _combine(results):
    outs = np.stack([np.asarray(r["out"], dtype=np.float64) for r in results])
    s = outs.sum(axis=(0, 1))  # [4]: sum|d|, sum r, sum lnS, sum lnG
    l1 = s[0] / N_TOT
    sum_lvr = s[2] - s[3] + N_TOT * LN6
    kl = 0.5 * (s[1] - sum_lvr - N_TOT) / N_TOT
    return np.float32(l1 + kl)


def kernel(x, z, Wm, bm, temb_w, t):
    global _built, LAST_RESULTS
    from concourse.bass_utils import run_bass_kernel_spmd

    if _built is None:
        _built = _build_nc()
    nc = _built

    in_maps = _prep_inputs(x, z, Wm, bm, temb_w, t)
    trace = bool(os.environ.get("BASS_TRACE"))
    res = run_bass_kernel_spmd(nc, in_maps, core_ids=list(range(N_CORES)), trace=trace)
    LAST_RESULTS = res
    return _combine(res.results)
